# revision 19
# baseline (speedup 1.0000x reference)
"""Trainium2 Bass kernel for ChannelMaxPool top-k masking (v4).

Reference computation:
  x: (B=32, C=512, H=128, W=128) f32
  scores[b,c] = max |x[b,c,:,:]|
  top-128 channels by score (jax.lax.top_k order: value desc, index asc)
  w[b,k] = exp(s_k) / sum_selected exp(s_j)
  y[b,k,:,:] = x[b, idx_k, :, :] * w[b,k]

Sharding: pure data-parallel, batch split across 8 NeuronCores
(4 samples per core), no communication.

Design notes (what profiling drove):
  * selection is rank-based: rank(c) = #{c': s' > s} + #{c'<c: s'==s}
    via comparison-count DVE ops against a PE-replicated score matrix
    B[p, c'] = s(c') -- exactly reproduces top_k tie order in ~14us of
    DVE (vs ~45us of serial MAX8 chains), with idx and w produced by
    tiny PE matmuls against the one-hot-of-rank matrix, so there are
    no SBUF->SBUF transpose DMAs on the critical path.
  * the gather is ONE indirect DMA per sample ([128, 16384], 64 KiB
    descriptors).  Indirect (SWDGE) descriptors only sustain ~24 GB/s
    per SDMA engine, so fewer+bigger descriptors beat many small ones.
  * stores are plain HWDGE DMAs (352 GB/s) in two halves; indirect
    scatter stores were tried and lose: every scatter that writes y
    serializes against the previous one (~4us completion wait each)
    because the tile framework must treat whole-tensor indirect
    writes as conflicting.
  * each sample's stores are deferred and emitted after the NEXT
    sample's load issues, so they execute inside that sample's
    selection window (the only DMA-idle window in steady state).
  * loads stream on Sync; the last tile of each sample is split 4x so
    the final absmax reduce (DVE, 1x-rate) gets off the critical path
    sooner.
"""

import numpy as np

B, C, H, W = 32, 512, 128, 128
S = H * W
K = 128
N_CORES = 8
BL = B // N_CORES

CCH = C // 128           # 4 channel groups of 128
ST_W = 4096              # streamed load tile width
NST = S // ST_W          # 4 stream tiles per group
NH = 2                   # store halves (8192 wide)
HW_ = S // NH            # 8192
FINE = 4                 # sub-splits of the very last tile per sample


def _build_nc():
    import concourse.bass as bass
    import concourse.mybir as mybir
    from concourse import bacc
    from concourse.masks import make_identity
    from concourse.tile import TileContext

    f32 = mybir.dt.float32
    i32 = mybir.dt.int32
    Alu = mybir.AluOpType
    Act = mybir.ActivationFunctionType

    nc = bacc.Bacc()
    x = nc.dram_tensor("x", [BL, C, S], f32, kind="ExternalInput")
    y = nc.dram_tensor("y", [BL, K, S], f32, kind="ExternalOutput")

    x2 = x[:].rearrange("b c (h s) -> (b c h) s", h=2)  # rows of 8192 (32 KiB)

    with TileContext(nc) as tc:
        with (
            tc.tile_pool(name="load", bufs=3) as load_pool,
            tc.tile_pool(name="gather", bufs=2) as gather_pool,
            tc.tile_pool(name="cmp", bufs=2) as cmp_pool,
            tc.tile_pool(name="big1", bufs=1) as big1,
            tc.tile_pool(name="small", bufs=2) as small,
            tc.tile_pool(name="const", bufs=1) as cpool,
            tc.psum_pool(name="psum", bufs=1) as psum,
        ):
            # ---------------- constants ----------------
            identity = cpool.tile([128, 128], f32, tag="identity")
            make_identity(nc, identity[:])

            ones4 = cpool.tile([CCH, 128], f32, tag="ones4")
            nc.vector.memset(ones4[:], 1.0)
            onescol = cpool.tile([128, 1], f32, tag="onescol")
            nc.vector.memset(onescol[:], 1.0)

            def iota_f32(tag, shape, pattern, cm):
                ti = cpool.tile(shape, i32, tag=tag + "_i")
                nc.gpsimd.iota(ti[:], pattern=pattern, base=0,
                               channel_multiplier=cm)
                tf = cpool.tile(shape, f32, tag=tag)
                nc.vector.tensor_copy(tf[:], ti[:])
                return tf

            iotaH2 = iota_f32("iotaH2", [128, 2], [[1, 2]], 0)
            cvals = iota_f32("cvals", [128, CCH], [[128, CCH]], 1)
            iotaQ = iota_f32("iotaQ", [128, 128], [[1, 128]], 0)

            # M[p, g, q'] = 1.0 if q' < 128g + p else 0.0
            mlt = cpool.tile([128, CCH, C], f32, tag="mlt")
            nc.gpsimd.memset(mlt[:], 1.0)
            nc.gpsimd.affine_select(
                out=mlt[:], in_=mlt[:], compare_op=Alu.is_gt, fill=0.0,
                base=0, pattern=[[128, CCH], [-1, C]], channel_multiplier=1,
            )
            # blkmask[g', g, q] = 1.0 if g == g'
            blkmask = cpool.tile([CCH, CCH, 128], f32, tag="blkmask")
            nc.gpsimd.memset(blkmask[:], 1.0)
            nc.gpsimd.affine_select(
                out=blkmask[:], in_=blkmask[:], compare_op=Alu.is_equal,
                fill=0.0, base=0, pattern=[[-1, CCH], [0, 128]],
                channel_multiplier=1,
            )

            deferred = []   # (dest-slice, tile) plain stores of sample b-1

            for b in range(BL):
                # ---- pass 1: per-channel absmax (all groups stream) ----
                n_cols = CCH * NST + FINE - 1
                partials = small.tile([128, n_cols], f32, tag="partials")
                col = 0
                for ci in range(CCH):
                    for t in range(NST):
                        last_tile = (ci == CCH - 1 and t == NST - 1)
                        sub = FINE if last_tile else 1
                        sw = ST_W // sub
                        for u in range(sub):
                            tile_in = load_pool.tile([128, ST_W], f32,
                                                     tag="ld")
                            s0 = t * ST_W + u * sw
                            nc.sync.dma_start(
                                out=tile_in[:, :sw],
                                in_=x[b, ci * 128:(ci + 1) * 128,
                                      s0:s0 + sw],
                            )
                            nc.vector.tensor_reduce(
                                out=partials[:, col:col + 1],
                                in_=tile_in[:, :sw],
                                axis=mybir.AxisListType.X,
                                op=Alu.max,
                                apply_absolute_value=True,
                            )
                            col += 1

                scores_col = small.tile([128, CCH], f32, tag="scores_col")
                nc.vector.tensor_reduce(
                    out=scores_col[:, :CCH - 1],
                    in_=partials[:, :(CCH - 1) * NST].rearrange(
                        "p (g t) -> p g t", t=NST),
                    axis=mybir.AxisListType.X,
                    op=Alu.max,
                )
                nc.vector.tensor_reduce(
                    out=scores_col[:, CCH - 1:CCH],
                    in_=partials[:, None, (CCH - 1) * NST:n_cols],
                    axis=mybir.AxisListType.X,
                    op=Alu.max,
                )

                # stores of the previous sample are pinned to execute
                # inside this sample's selection window: a tiny in-place
                # DVE op on each store's source, ordered after this
                # sample's scores, gates the store issue
                for dst, gsrc in deferred:
                    nc.vector.tensor_scalar(
                        out=gsrc[0:1, 0:1], in0=gsrc[0:1, 0:1],
                        scalar1=0.0, scalar2=None, op0=Alu.add,
                    )
                    nc.sync.dma_start(out=dst, in_=gsrc)
                deferred = []

                # ---- replicate scores to all partitions via PE ----
                sc_t_ps = psum.tile([CCH, 128], f32, tag="sc_t")
                nc.tensor.transpose(
                    out=sc_t_ps[:], in_=scores_col[:], identity=identity[:])
                sc_t = small.tile([CCH, 128], f32, tag="sc_t_sb")
                nc.vector.tensor_copy(sc_t[:], sc_t_ps[:])
                rhs_blk = small.tile([CCH, CCH, 128], f32, tag="rhs_blk")
                nc.vector.tensor_tensor(
                    out=rhs_blk[:],
                    in0=sc_t[:, None, :].to_broadcast([CCH, CCH, 128]),
                    in1=blkmask[:],
                    op=Alu.mult,
                )
                b_ps = psum.tile([128, C], f32, tag="B")
                nc.tensor.matmul(
                    out=b_ps[:], lhsT=ones4[:], rhs=rhs_blk[:],
                    start=True, stop=True,
                )
                b_sb = big1.tile([128, C], f32, tag="b_sb")
                nc.vector.tensor_copy(b_sb[:], b_ps[:])

                # ---- rank(c) by comparison counting ----
                rank_col = small.tile([128, CCH], f32, tag="rank_col")
                r2 = small.tile([128, CCH], f32, tag="r2")
                r1 = small.tile([128, CCH], f32, tag="r1")
                for g in range(CCH):
                    cmp = cmp_pool.tile([128, C], f32, tag="cmp")
                    nc.vector.tensor_tensor(
                        out=cmp[:],
                        in0=b_sb[:],
                        in1=scores_col[:, g:g + 1].to_broadcast([128, C]),
                        op=Alu.is_equal,
                    )
                    cmp2 = cmp_pool.tile([128, C], f32, tag="cmp")
                    nc.vector.tensor_tensor(
                        out=cmp2[:], in0=cmp[:], in1=mlt[:, g, :],
                        op=Alu.mult,
                    )
                    nc.vector.reduce_sum(
                        out=r2[:, g:g + 1], in_=cmp2[:],
                        axis=mybir.AxisListType.X,
                    )
                    cmp3 = cmp_pool.tile([128, C], f32, tag="cmp")
                    nc.vector.tensor_tensor(
                        out=cmp3[:],
                        in0=b_sb[:],
                        in1=scores_col[:, g:g + 1].to_broadcast([128, C]),
                        op=Alu.is_gt,
                    )
                    nc.vector.reduce_sum(
                        out=r1[:, g:g + 1], in_=cmp3[:],
                        axis=mybir.AxisListType.X,
                    )
                nc.vector.tensor_tensor(
                    out=rank_col[:], in0=r1[:], in1=r2[:], op=Alu.add)

                # ---- softmax weights over the selected set ----
                e_col = small.tile([128, CCH], f32, tag="e_col")
                nc.scalar.activation(
                    out=e_col[:], in_=scores_col[:], func=Act.Exp,
                    bias=0.0, scale=1.0,
                )
                es0 = small.tile([128, CCH], f32, tag="es0")
                nc.vector.scalar_tensor_tensor(
                    out=es0[:], in0=rank_col[:], scalar=float(K),
                    in1=e_col[:], op0=Alu.is_lt, op1=Alu.mult,
                )
                esum = small.tile([128, 1], f32, tag="esum")
                nc.vector.reduce_sum(
                    out=esum[:], in_=es0[:], axis=mybir.AxisListType.X)
                z_ps = psum.tile([128, 4], f32, tag="zsmall")
                nc.tensor.matmul(
                    out=z_ps[0:1, 0:1], lhsT=onescol[:], rhs=esum[:],
                    start=True, stop=True,
                )
                z_sb = small.tile([1, 1], f32, tag="z_sb")
                nc.vector.tensor_copy(z_sb[:], z_ps[0:1, 0:1])
                zrep_ps = psum.tile([128, 1], f32, tag="zrep")
                nc.tensor.matmul(
                    out=zrep_ps[:], lhsT=ones4[0:1, :], rhs=z_sb[:],
                    start=True, stop=True,
                )
                zrep_sb = small.tile([128, 1], f32, tag="zrep_sb")
                nc.vector.tensor_copy(zrep_sb[:], zrep_ps[:])
                zinv = small.tile([128, 1], f32, tag="zinv")
                nc.vector.reciprocal(zinv[:], zrep_sb[:])

                # ---- one-hot of rank -> idx and w via PE ----
                oh = big1.tile([128, C], f32, tag="oh")
                for g in range(CCH):
                    nc.vector.tensor_tensor(
                        out=oh[:, g * 128:(g + 1) * 128],
                        in0=iotaQ[:],
                        in1=rank_col[:, g:g + 1].to_broadcast([128, 128]),
                        op=Alu.is_equal,
                    )
                rhs2 = small.tile([128, CCH, 2], f32, tag="rhs2")
                nc.vector.tensor_copy(rhs2[:, :, 0], cvals[:])
                nc.vector.tensor_copy(rhs2[:, :, 1], es0[:])
                idxw_ps = psum.tile([128, 2], f32, tag="idxw")
                for g in range(CCH):
                    nc.tensor.matmul(
                        out=idxw_ps[:],
                        lhsT=oh[:, g * 128:(g + 1) * 128],
                        rhs=rhs2[:, g, :],
                        start=(g == 0),
                        stop=(g == CCH - 1),
                    )
                idxw_sb = small.tile([128, 2], f32, tag="idxw_sb")
                nc.vector.tensor_copy(idxw_sb[:], idxw_ps[:])
                w_sb = small.tile([128, 1], f32, tag="w_sb")
                nc.vector.tensor_tensor(
                    out=w_sb[:], in0=idxw_sb[:, 1:2], in1=zinv[:],
                    op=Alu.mult,
                )
                # gather offsets: (512b + idx)*2 + h (32 KiB rows of x2)
                idx2_f = small.tile([128, 2], f32, tag="idx2_f")
                nc.vector.scalar_tensor_tensor(
                    out=idx2_f[:],
                    in0=idxw_sb[:, 0:1].to_broadcast([128, 2]),
                    scalar=2.0,
                    in1=iotaH2[:],
                    op0=Alu.mult, op1=Alu.add,
                )
                idx2_i = small.tile([128, 2], i32, tag="idx2_i")
                nc.vector.tensor_scalar(
                    out=idx2_i[:], in0=idx2_f[:],
                    scalar1=float(b * C * 2), scalar2=None, op0=Alu.add,
                )

                # ---- gather halves, scale + store quarters ----
                gh = gather_pool.tile([128, S], f32, tag="gh")
                for h in range(2):
                    nc.gpsimd.indirect_dma_start(
                        out=gh[:, h * 8192:(h + 1) * 8192],
                        out_offset=None,
                        in_=x2,
                        in_offset=bass.IndirectOffsetOnAxis(
                            ap=idx2_i[:, h:h + 1], axis=0),
                    )
                QW = S // 4
                for q in range(4):
                    nc.vector.tensor_scalar(
                        out=gh[:, q * QW:(q + 1) * QW],
                        in0=gh[:, q * QW:(q + 1) * QW],
                        scalar1=w_sb[:, 0:1],
                        scalar2=None, op0=Alu.mult,
                    )
                    dst = y[b, :, q * QW:(q + 1) * QW]
                    if b < BL - 1:
                        deferred.append((dst, gh[:, q * QW:(q + 1) * QW]))
                    else:
                        nc.sync.dma_start(
                            out=dst, in_=gh[:, q * QW:(q + 1) * QW])
    if not nc.is_finalized():
        nc.finalize()
    return nc


_NC_CACHE = None


def _get_nc():
    global _NC_CACHE
    if _NC_CACHE is None:
        _NC_CACHE = _build_nc()
    return _NC_CACHE


def _run(x, trace=False):
    from concourse.bass_utils import run_bass_kernel_spmd

    nc = _get_nc()
    xr = np.ascontiguousarray(x, dtype=np.float32).reshape(N_CORES, BL, C, S)
    in_maps = [{"x": xr[c]} for c in range(N_CORES)]
    res = run_bass_kernel_spmd(nc, in_maps, list(range(N_CORES)), trace=trace)
    out = np.empty((B, K, H, W), dtype=np.float32)
    for c in range(N_CORES):
        out[c * BL:(c + 1) * BL] = res.results[c]["y"].reshape(BL, K, H, W)
    return out, res


def kernel(x):
    out, _ = _run(x, trace=False)
    return out
